# revision 2
# baseline (speedup 1.0000x reference)
"""Multi-head attention (B=4, N=2048, D=1024, H=16, DH=64) on 8 TRN2 NeuronCores.

Sharding: core c <- (batch b = c//2, head-group g = c%2 of 8 heads).
  Each core computes its 8 heads' attention for its batch and the partial
  output projection (row-split Wo). Host sums the 2 partials per batch and
  adds the bias (the unshard step of tensor parallelism).

Device-side layout (all matmul operands bf16, fp32 PSUM accumulation):
  xT [D, N] per batch (host pre-transpose, a sharding-layout choice)
  Q^T, K^T [512, N]; scores computed transposed S^T = K Q^T so softmax
  normalizer comes free from a ones-column in V; exp on ACT engine from
  4-bank PSUM tiles; PV gives O^T; out-proj gives natural [N, D] partials.
"""

import sys
from contextlib import ExitStack

import numpy as np

sys.path.insert(0, "/opt/trn_rl_repo")

import concourse.bass as bass
import concourse.mybir as mybir
import concourse.tile as tile
from concourse import bacc
from concourse.bass_utils import run_bass_kernel_spmd

F32 = mybir.dt.float32
F32R = mybir.dt.float32r
BF16 = mybir.dt.bfloat16
EXP = mybir.ActivationFunctionType.Exp

B, N_TOK, D, H_TOT, DH = 4, 2048, 1024, 16, 64
H = 8  # heads per core
HD = H * DH  # 512
SCALE = DH ** -0.5
N_CORES = 8


def emit_attention(ctx, tc, xt, wq, wk, wv, wo, out, scale):
    """One core's shard. xt [D,N] f32; wq/wk/wv [D,HD]; wo [HD,D]; out [N,D]."""
    nc = tc.nc
    D_, N = xt.shape
    HD_ = wq.shape[1]
    H_ = HD_ // 64
    KC = D_ // 128
    TC = N // 128
    TB = N // 512
    MC = HD_ // 128
    NB = D_ // 512

    p_tmp = ctx.enter_context(tc.tile_pool(name="tmp", bufs=3))
    p_xt = ctx.enter_context(tc.tile_pool(name="xt", bufs=2 * KC))
    p_w = ctx.enter_context(tc.tile_pool(name="w", bufs=3 * KC))
    p_wo = ctx.enter_context(tc.tile_pool(name="wo", bufs=MC * NB))
    p_qt = ctx.enter_context(tc.tile_pool(name="qt", bufs=MC))
    p_kt = ctx.enter_context(tc.tile_pool(name="kt", bufs=MC))
    p_v = ctx.enter_context(tc.tile_pool(name="v", bufs=TC))
    p_pt = ctx.enter_context(tc.tile_pool(name="pt", bufs=6))
    p_ot = ctx.enter_context(tc.tile_pool(name="ot", bufs=MC))
    p_small = ctx.enter_context(tc.tile_pool(name="small", bufs=3))
    p_stage = ctx.enter_context(tc.tile_pool(name="stage", bufs=3))

    ps_s = ctx.enter_context(tc.tile_pool(name="ps_s", bufs=1, space="PSUM"))
    ps_o = ctx.enter_context(tc.tile_pool(name="ps_o", bufs=2, space="PSUM"))
    ps_mm = ctx.enter_context(tc.tile_pool(name="ps_mm", bufs=2, space="PSUM"))

    # ---- weights: load + cast to bf16 (resident) ----
    w_t = {}
    for nm, w_dram in (("q", wq), ("k", wk), ("v", wv)):
        for k in range(KC):
            t_in = p_tmp.tile([128, HD_], F32, name=f"wt_{nm}{k}", tag="tmp")
            nc.sync.dma_start(t_in[:], w_dram[k * 128 : (k + 1) * 128, :])
            t_bf = p_w.tile([128, HD_], BF16, name=f"w_{nm}{k}", tag="w")
            nc.vector.tensor_copy(t_bf[:], t_in[:])
            w_t[(nm, k)] = t_bf
    wo_t = {}
    for kc in range(MC):
        t_in = p_tmp.tile([128, D_], F32, name=f"wot{kc}", tag="tmp")
        nc.sync.dma_start(t_in[:], wo[kc * 128 : (kc + 1) * 128, :])
        for nb in range(NB):
            t_bf = p_wo.tile([128, 512], BF16, name=f"wo{kc}_{nb}", tag="wo")
            nc.vector.tensor_copy(t_bf[:], t_in[:, nb * 512 : (nb + 1) * 512])
            wo_t[(kc, nb)] = t_bf

    # ---- xT: load + cast, per (k-chunk, 512-token-block) slices ----
    xt_b = {}
    for tb in range(TB):
        for k in range(KC):
            t_in = p_tmp.tile([128, 512], F32, name=f"xt_in{k}_{tb}", tag="xtmp")
            nc.sync.dma_start(
                t_in[:], xt[k * 128 : (k + 1) * 128, tb * 512 : (tb + 1) * 512]
            )
            t_bf = p_xt.tile([128, 512], BF16, name=f"xt{k}_{tb}", tag="xt")
            nc.vector.tensor_copy(t_bf[:], t_in[:])
            xt_b[(k, tb)] = t_bf

    # ---- QKV projections ----
    qt_tiles = [p_qt.tile([128, N], BF16, name=f"qt{m}", tag="qt") for m in range(MC)]
    kt_tiles = [p_kt.tile([128, N], BF16, name=f"kt{m}", tag="kt") for m in range(MC)]
    v_tiles = []
    for tm in range(TC):
        vt = p_v.tile([128, H_ * 65], BF16, name=f"v{tm}", tag="v")
        nc.vector.memset(vt[:], 1.0)
        v_tiles.append(vt)

    for tb in range(TB):
        for nm, out_tiles in (("q", qt_tiles), ("k", kt_tiles)):
            for m in range(MC):
                ps = ps_mm.tile([128, 512], F32, name=f"ps{nm}{m}_{tb}", tag="ps_mm")
                for k in range(KC):
                    nc.tensor.matmul(
                        ps[:],
                        w_t[(nm, k)][:, m * 128 : (m + 1) * 128],
                        xt_b[(k, tb)][:],
                        start=(k == 0),
                        stop=(k == KC - 1),
                    )
                nc.vector.tensor_copy(
                    out_tiles[m][:, tb * 512 : (tb + 1) * 512], ps[:]
                )
        for u in range(4):  # token chunks within this tb
            tm = tb * 4 + u
            ps = ps_mm.tile([128, HD_], F32, name=f"psv{tm}", tag="ps_mm")
            for k in range(KC):
                nc.tensor.matmul(
                    ps[:],
                    xt_b[(k, tb)][:, u * 128 : (u + 1) * 128],
                    w_t[("v", k)][:],
                    start=(k == 0),
                    stop=(k == KC - 1),
                )
            dst = v_tiles[tm][:].rearrange("p (h c) -> p h c", h=H_)[:, :, 0:64]
            src = ps[:].rearrange("p (h c) -> p h c", h=H_)
            nc.vector.tensor_copy(dst, src)

    # ---- heads ----
    ot_tiles = [p_ot.tile([128, N], BF16, name=f"ot{m}", tag="ot") for m in range(MC)]
    ones_f = p_small.tile([1, 64], F32, name="ones_f", tag="ones_f")
    nc.vector.memset(ones_f[:], 1.0)
    ones_r = p_small.tile([1, 64], F32R, name="ones_r", tag="ones_r")
    nc.vector.tensor_copy(ones_r[:], ones_f[:])

    for h in range(H_):
        mh, po = h // 2, (h % 2) * 64
        for ib in range(TB):
            qt_sl = qt_tiles[mh][po : po + 64, ib * 512 : (ib + 1) * 512]
            n_grp = (TC + 3) // 4
            pt_grp = []
            for g in range(n_grp):
                u_n = min(4, TC - g * 4)
                pss = ps_s.tile([128, 2048], F32, name=f"ps_s{h}_{ib}_{g}", tag="ps_s")
                for u in range(u_n):
                    jc = g * 4 + u
                    nc.tensor.matmul(
                        pss[:, u * 512 : (u + 1) * 512],
                        kt_tiles[mh][po : po + 64, jc * 128 : (jc + 1) * 128],
                        qt_sl,
                        start=True,
                        stop=True,
                    )
                ptg = p_pt.tile([128, 2048], BF16, name=f"pt{h}_{ib}_{g}", tag="pt")
                nc.scalar.activation(
                    ptg[:, 0 : u_n * 512], pss[:, 0 : u_n * 512], EXP, scale=scale
                )
                pt_grp.append(ptg)

            pso = ps_o.tile([128, 512], F32, name=f"ps_o{h}_{ib}", tag="ps_o")
            for jc in range(TC):
                nc.tensor.matmul(
                    pso[0:65, :],
                    v_tiles[jc][:, h * 65 : h * 65 + 65],
                    pt_grp[jc // 4][:, (jc % 4) * 512 : (jc % 4 + 1) * 512],
                    start=(jc == 0),
                    stop=(jc == TC - 1),
                )
            rsum = p_small.tile([1, 512], F32, name=f"rsum{h}_{ib}", tag="rsum")
            nc.vector.tensor_copy(rsum[:], pso[64:65, :])
            rcp = p_small.tile([1, 512], F32, name=f"rcp{h}_{ib}", tag="rcp")
            nc.vector.reciprocal_approx_fast(rcp[:], rsum[:])
            rcp_r = p_small.tile([1, 512], F32R, name=f"rcpr{h}_{ib}", tag="rcpr")
            nc.vector.tensor_copy(rcp_r[:], rcp[:])
            psb = ps_mm.tile([128, 512], F32, name=f"ps_b{h}_{ib}", tag="ps_mm")
            nc.tensor.matmul(psb[0:64, :], ones_r[:], rcp_r[:], start=True, stop=True)
            rcp_bc = p_small.tile([64, 512], F32, name=f"rcpb{h}_{ib}", tag="rcpb")
            nc.vector.tensor_copy(rcp_bc[:], psb[0:64, :])
            nc.vector.tensor_tensor(
                ot_tiles[mh][po : po + 64, ib * 512 : (ib + 1) * 512],
                pso[0:64, :],
                rcp_bc[:],
                op=mybir.AluOpType.mult,
            )

    # ---- output projection ----
    for tm in range(TC):
        stage = p_stage.tile([128, D_], F32, name=f"stg{tm}", tag="stage")
        for nb in range(NB):
            ps = ps_mm.tile([128, 512], F32, name=f"ps_p{tm}_{nb}", tag="ps_mm")
            for kc in range(MC):
                nc.tensor.matmul(
                    ps[:],
                    ot_tiles[kc][:, tm * 128 : (tm + 1) * 128],
                    wo_t[(kc, nb)][:],
                    start=(kc == 0),
                    stop=(kc == MC - 1),
                )
            nc.vector.tensor_copy(stage[:, nb * 512 : (nb + 1) * 512], ps[:])
        nc.sync.dma_start(out[tm * 128 : (tm + 1) * 128, :], stage[:])


_NC_CACHE = {}


def build_nc():
    if "nc" in _NC_CACHE:
        return _NC_CACHE["nc"]
    nc = bacc.Bacc("TRN2", target_bir_lowering=False, debug=False, num_devices=N_CORES)
    xt = nc.dram_tensor("xt", [D, N_TOK], F32, kind="ExternalInput")
    wq = nc.dram_tensor("wq", [D, HD], F32, kind="ExternalInput")
    wk = nc.dram_tensor("wk", [D, HD], F32, kind="ExternalInput")
    wv = nc.dram_tensor("wv", [D, HD], F32, kind="ExternalInput")
    wo = nc.dram_tensor("wo", [HD, D], F32, kind="ExternalInput")
    out = nc.dram_tensor("out", [N_TOK, D], F32, kind="ExternalOutput")
    with tile.TileContext(nc) as tc:
        with ExitStack() as ctx:
            emit_attention(
                ctx, tc, xt.ap(), wq.ap(), wk.ap(), wv.ap(), wo.ap(), out.ap(), SCALE
            )
    nc.compile()
    _NC_CACHE["nc"] = nc
    return nc


def kernel(x, Wq, Wk, Wv, Wo, bo, _trace=False, _trace_kwargs=None):
    assert x.shape == (B, N_TOK, D)
    nc = build_nc()
    in_maps = []
    for c in range(N_CORES):
        b, g = c // 2, c % 2
        in_maps.append(
            {
                "xt": np.ascontiguousarray(x[b].T).astype(np.float32),
                "wq": np.ascontiguousarray(Wq[:, g * HD : (g + 1) * HD]),
                "wk": np.ascontiguousarray(Wk[:, g * HD : (g + 1) * HD]),
                "wv": np.ascontiguousarray(Wv[:, g * HD : (g + 1) * HD]),
                "wo": np.ascontiguousarray(Wo[g * HD : (g + 1) * HD, :]),
            }
        )
    res = run_bass_kernel_spmd(
        nc,
        in_maps,
        core_ids=list(range(N_CORES)),
        trace=_trace,
        **(_trace_kwargs or {}),
    )
    out = np.empty((B, N_TOK, D), dtype=np.float32)
    for b in range(B):
        out[b] = res.results[2 * b]["out"] + res.results[2 * b + 1]["out"] + bo
    if _trace:
        kernel.last_results = res
    return out


# revision 6
# speedup vs baseline: 1.3556x; 1.3556x over previous
"""Multi-head attention (B=4, N=2048, D=1024, H=16, DH=64) on 8 TRN2 NeuronCores.

Sharding: core c <- (batch b = c//2, head-group g = c%2 of 8 heads).
  Each core computes its 8 heads' attention for its batch and the partial
  output projection (row-split Wo). Host sums the 2 partials per batch and
  adds the bias (the unshard step of tensor parallelism).

Device design (v3, informed by trace analysis of earlier versions):
  - all matmul operands bf16; every matmul N=512 (one PSUM bank per write);
    warm PE issues these back-to-back at ~216ns with LDWEIGHTS hidden.
  - scores computed transposed S^T[j,i] = K_h Q_h^T with K=64 stationary
    slices (head h lives at partition offset (h%2)*64 of its 128-row chunk).
  - exp on ACT from [128,1024] PSUM tiles (pss pool, 2 bufs) -> bf16 P^T.
  - PV accumulates O^T plus a ones-column softmax denominator row into one
    [128,2048] PSUM tile per head (pso pool, 1 buf). 4+4 banks total.
  - softmax normalization off the critical path: rowsum -> fast reciprocal
    -> DRAM bounce -> 0-stride broadcast read -> in-place DVE multiply.
  - V-first ordering so the per-head pipeline starts after only V + QK(m=0).
"""

import sys
from contextlib import ExitStack

import numpy as np

sys.path.insert(0, "/opt/trn_rl_repo")

import concourse.bass as bass
import concourse.mybir as mybir
import concourse.tile as tile
from concourse import bacc
from concourse.bass_utils import run_bass_kernel_spmd

F32 = mybir.dt.float32
BF16 = mybir.dt.bfloat16
EXP = mybir.ActivationFunctionType.Exp

B, N_TOK, D, H_TOT, DH = 4, 2048, 1024, 16, 64
H = 8  # heads per core
HD = H * DH  # 512
SCALE = DH ** -0.5
N_CORES = 8


def emit_attention(ctx, tc, xt, wq, wk, wv, wo, out, rcp_dram, scale):
    """One core's shard. xt [D,N] f32; wq/wk/wv [D,HD]; wo [HD,D]; out [N,D];
    rcp_dram [H, N] f32 internal scratch for softmax-denominator broadcast."""
    nc = tc.nc
    D_, N = xt.shape
    HD_ = wq.shape[1]
    H_ = HD_ // 64
    KC = D_ // 128   # contraction chunks over model dim
    TC = N // 128    # token chunks (j-chunks)
    IB = N // 512    # 512-wide moving blocks
    MC = HD_ // 128  # chunks over per-core head dim
    NB = D_ // 512   # output column blocks

    p_tmp = ctx.enter_context(tc.tile_pool(name="tmp", bufs=3))
    p_xt = ctx.enter_context(tc.tile_pool(name="xt", bufs=KC))
    p_w = ctx.enter_context(tc.tile_pool(name="w", bufs=2 * KC))
    p_wo = ctx.enter_context(tc.tile_pool(name="wo", bufs=MC))
    p_qt = ctx.enter_context(tc.tile_pool(name="qt", bufs=MC))
    p_kt = ctx.enter_context(tc.tile_pool(name="kt", bufs=MC))
    p_v = ctx.enter_context(tc.tile_pool(name="v", bufs=TC))
    p_pt = ctx.enter_context(tc.tile_pool(name="pt", bufs=3))
    p_ot = ctx.enter_context(tc.tile_pool(name="ot", bufs=MC))
    p_nrm = ctx.enter_context(tc.tile_pool(name="nrm", bufs=1))
    p_stage = ctx.enter_context(tc.tile_pool(name="stage", bufs=2))

    ps_big = ctx.enter_context(tc.tile_pool(name="ps_big", bufs=1, space="PSUM"))
    ps_sml = ctx.enter_context(tc.tile_pool(name="ps_sml", bufs=2, space="PSUM"))

    # ---- load + cast weights (wv first: V phase runs first) ----
    w_t = {}
    for nm, w_dram in (("v", wv), ("q", wq), ("k", wk)):
        for k in range(KC):
            t_in = p_tmp.tile([128, HD_], F32, name=f"wt_{nm}{k}", tag="tmp")
            nc.sync.dma_start(t_in[:], w_dram[k * 128 : (k + 1) * 128, :])
            t_bf = p_w.tile([128, HD_], BF16, name=f"w_{nm}{k}", tag="w")
            nc.vector.tensor_copy(t_bf[:], t_in[:])
            w_t[(nm, k)] = t_bf
    wo_t = {}
    for kc in range(MC):
        t_in = p_tmp.tile([128, D_], F32, name=f"wot{kc}", tag="tmp")
        nc.sync.dma_start(t_in[:], wo[kc * 128 : (kc + 1) * 128, :])
        t_bf = p_wo.tile([128, D_], BF16, name=f"wo{kc}", tag="wo")
        nc.vector.tensor_copy(t_bf[:], t_in[:])
        wo_t[kc] = t_bf

    # ---- xT: load f32 halves, cast to resident bf16 [128, N] chunks ----
    xt_t = []
    for k in range(KC):
        t_bf = p_xt.tile([128, N], BF16, name=f"xt{k}", tag="xt")
        for hf in range(N // 1024):
            t_in = p_tmp.tile([128, 1024], F32, name=f"xin{k}_{hf}", tag="xtmp")
            nc.sync.dma_start(
                t_in[:], xt[k * 128 : (k + 1) * 128, hf * 1024 : (hf + 1) * 1024]
            )
            nc.vector.tensor_copy(t_bf[:, hf * 1024 : (hf + 1) * 1024], t_in[:])
        xt_t.append(t_bf)

    # ---- V: natural [tokens, dh] -> v_aug tiles [128, H*65] (ones col) ----
    v_tiles = []
    for tm in range(TC):
        vt = p_v.tile([128, H_ * 65], BF16, name=f"v{tm}", tag="v")
        nc.vector.memset(vt[:], 1.0)
        v_tiles.append(vt)
    for tp in range(TC // 2):
        ps = ps_sml.tile([128, 2 * HD_], F32, name=f"psv{tp}", tag="ps_sml")
        for k in range(KC):
            for u in range(2):
                tm = tp * 2 + u
                nc.tensor.matmul(
                    ps[:, u * HD_ : (u + 1) * HD_],
                    xt_t[k][:, tm * 128 : (tm + 1) * 128],
                    w_t[("v", k)][:],
                    start=(k == 0),
                    stop=(k == KC - 1),
                )
        for u in range(2):
            tm = tp * 2 + u
            dst = v_tiles[tm][:].rearrange("p (h c) -> p h c", h=H_)[:, :, 0:64]
            src = ps[:, u * HD_ : (u + 1) * HD_].rearrange("p (h c) -> p h c", h=H_)
            nc.vector.tensor_copy(dst, src)

    # ---- Q^T, K^T: [HD, N] bf16 (w stationary, xt moving N=512) ----
    qt_tiles = [p_qt.tile([128, N], BF16, name=f"qt{m}", tag="qt") for m in range(MC)]
    kt_tiles = [p_kt.tile([128, N], BF16, name=f"kt{m}", tag="kt") for m in range(MC)]
    for m in range(MC):
        psq = ps_big.tile([128, N], F32, name=f"psq{m}", tag="ps_big")
        for k in range(KC):
            for ib in range(IB):
                nc.tensor.matmul(
                    psq[:, ib * 512 : (ib + 1) * 512],
                    w_t[("q", k)][:, m * 128 : (m + 1) * 128],
                    xt_t[k][:, ib * 512 : (ib + 1) * 512],
                    start=(k == 0),
                    stop=(k == KC - 1),
                )
        nc.vector.tensor_copy(qt_tiles[m][:], psq[:])
        for half in range(IB // 2):
            psk = ps_sml.tile([128, 1024], F32, name=f"psk{m}_{half}", tag="ps_sml")
            for k in range(KC):
                for u in range(2):
                    ib = half * 2 + u
                    nc.tensor.matmul(
                        psk[:, u * 512 : (u + 1) * 512],
                        w_t[("k", k)][:, m * 128 : (m + 1) * 128],
                        xt_t[k][:, ib * 512 : (ib + 1) * 512],
                        start=(k == 0),
                        stop=(k == KC - 1),
                    )
            nc.vector.tensor_copy(
                kt_tiles[m][:, half * 1024 : (half + 1) * 1024], psk[:]
            )

    # ---- heads: scores -> exp -> PV -> (rsum, evac, recip->DRAM bcast) ----
    ot_tiles = [p_ot.tile([128, N], BF16, name=f"ot{m}", tag="ot") for m in range(MC)]

    for h in range(H_):
        mh, po = h // 2, (h % 2) * 64
        pso = ps_big.tile([128, N], F32, name=f"pso{h}", tag="ps_big")
        for jc in range(TC):
            kt_sl = kt_tiles[mh][po : po + 64, jc * 128 : (jc + 1) * 128]
            ptg = p_pt.tile([128, N], BF16, name=f"pt{h}_{jc}", tag="pt")
            for hf in range(IB // 2):
                pss = ps_sml.tile([128, 1024], F32, name=f"pss{h}_{jc}_{hf}", tag="ps_sml")
                for u in range(2):
                    nc.tensor.matmul(
                        pss[:, u * 512 : (u + 1) * 512],
                        kt_sl,
                        qt_tiles[mh][
                            po : po + 64, (hf * 2 + u) * 512 : (hf * 2 + u + 1) * 512
                        ],
                        start=True,
                        stop=True,
                    )
                nc.scalar.activation(
                    ptg[:, hf * 1024 : (hf + 1) * 1024], pss[:], EXP, scale=scale
                )
            for ib in range(IB):
                nc.tensor.matmul(
                    pso[0:65, ib * 512 : (ib + 1) * 512],
                    v_tiles[jc][:, h * 65 : h * 65 + 65],
                    ptg[:, ib * 512 : (ib + 1) * 512],
                    start=(jc == 0),
                    stop=(jc == TC - 1),
                )
        # rowsum -> recip -> DRAM (off critical path); O^T evac unnormalized
        rsum = p_nrm.tile([1, N], F32, name=f"rsum{h}", tag="rsum")
        nc.vector.tensor_copy(rsum[:], pso[64:65, :])
        nc.vector.tensor_copy(ot_tiles[mh][po : po + 64, :], pso[0:64, :])
        rcp = p_nrm.tile([1, N], F32, name=f"rcp{h}", tag="rcp")
        nc.vector.reciprocal_approx_fast(rcp[:], rsum[:])
        nc.sync.dma_start(rcp_dram[h : h + 1, :], rcp[:])
        # broadcast back over 64 partitions (0-stride DRAM read), normalize
        bc = p_nrm.tile([128, N], F32, name=f"bc{h}", tag="bc")
        nc.sync.dma_start(
            bc[po : po + 64, :], rcp_dram[h : h + 1, :].to_broadcast((64, N))
        )
        nc.vector.tensor_tensor(
            ot_tiles[mh][po : po + 64, :],
            ot_tiles[mh][po : po + 64, :],
            bc[po : po + 64, :],
            op=mybir.AluOpType.mult,
        )

    # ---- output projection: out[t, dout] (ot stationary, wo moving) ----
    for tm in range(TC):
        stage = p_stage.tile([128, D_], F32, name=f"stg{tm}", tag="stage")
        for half in range(NB // 2):
            ps = ps_sml.tile([128, 1024], F32, name=f"psp{tm}_{half}", tag="ps_sml")
            for kc in range(MC):
                for u in range(2):
                    nb = half * 2 + u
                    nc.tensor.matmul(
                        ps[:, u * 512 : (u + 1) * 512],
                        ot_tiles[kc][:, tm * 128 : (tm + 1) * 128],
                        wo_t[kc][:, nb * 512 : (nb + 1) * 512],
                        start=(kc == 0),
                        stop=(kc == MC - 1),
                    )
            nc.vector.tensor_copy(
                stage[:, half * 1024 : (half + 1) * 1024], ps[:]
            )
        nc.sync.dma_start(out[tm * 128 : (tm + 1) * 128, :], stage[:])


_NC_CACHE = {}


def build_nc():
    if "nc" in _NC_CACHE:
        return _NC_CACHE["nc"]
    nc = bacc.Bacc("TRN2", target_bir_lowering=False, debug=False, num_devices=N_CORES)
    xt = nc.dram_tensor("xt", [D, N_TOK], F32, kind="ExternalInput")
    wq = nc.dram_tensor("wq", [D, HD], F32, kind="ExternalInput")
    wk = nc.dram_tensor("wk", [D, HD], F32, kind="ExternalInput")
    wv = nc.dram_tensor("wv", [D, HD], F32, kind="ExternalInput")
    wo = nc.dram_tensor("wo", [HD, D], F32, kind="ExternalInput")
    out = nc.dram_tensor("out", [N_TOK, D], F32, kind="ExternalOutput")
    rcp_d = nc.dram_tensor("rcp_d", [H, N_TOK], F32, kind="Internal")
    with tile.TileContext(nc) as tc:
        with ExitStack() as ctx:
            emit_attention(
                ctx, tc, xt.ap(), wq.ap(), wk.ap(), wv.ap(), wo.ap(), out.ap(),
                rcp_d.ap(), SCALE,
            )
    nc.compile()
    _NC_CACHE["nc"] = nc
    return nc


def kernel(x, Wq, Wk, Wv, Wo, bo, _trace=False, _trace_kwargs=None):
    assert x.shape == (B, N_TOK, D)
    nc = build_nc()
    in_maps = []
    for c in range(N_CORES):
        b, g = c // 2, c % 2
        in_maps.append(
            {
                "xt": np.ascontiguousarray(x[b].T).astype(np.float32),
                "wq": np.ascontiguousarray(Wq[:, g * HD : (g + 1) * HD]),
                "wk": np.ascontiguousarray(Wk[:, g * HD : (g + 1) * HD]),
                "wv": np.ascontiguousarray(Wv[:, g * HD : (g + 1) * HD]),
                "wo": np.ascontiguousarray(Wo[g * HD : (g + 1) * HD, :]),
            }
        )
    res = run_bass_kernel_spmd(
        nc,
        in_maps,
        core_ids=list(range(N_CORES)),
        trace=_trace,
        **(_trace_kwargs or {}),
    )
    out = np.empty((B, N_TOK, D), dtype=np.float32)
    for b in range(B):
        out[b] = res.results[2 * b]["out"] + res.results[2 * b + 1]["out"] + bo
    if _trace:
        kernel.last_results = res
    return out


# revision 10
# speedup vs baseline: 1.4174x; 1.0457x over previous
"""Multi-head attention (B=4, N=2048, D=1024, H=16, DH=64) on 8 TRN2 NeuronCores.

Sharding: core c <- (batch b = c//2, head-group g = c%2 of 8 heads).
  Each core computes its 8 heads' attention for its batch and the partial
  output projection (row-split Wo). Host sums the 2 partials per batch and
  adds the bias (the unshard step of tensor parallelism).

Device design (v4):
  - all matmul operands bf16, every matmul N=512 out (one PSUM bank/write);
    warm PE issues back-to-back at ~216ns with LDWEIGHTS pulled ahead.
  - heads processed in two 1024-wide i-half passes; PSUM split into four
    [128,1024] pools (2 banks each): scores ping/pong, PV accumulator, and
    a filler pool so V / later QK projections / output projection matmuls
    can run inside the ACT-bound heads phase's PE slack.
  - emission order = QK(m=0), head 0, V, QK1, H1, H2, QK2, H3, H4, QK3,
    H5, H6, H7, outproj; the Tile scheduler back-fills PE gaps with the
    lower-priority filler work while exp paces the pipeline.
  - softmax normalization off the critical path: ones-column rowsums ->
    fast reciprocal -> DRAM bounce -> 0-stride broadcast read -> in-place
    DVE multiply on the unnormalized O^T.
"""

import sys
from contextlib import ExitStack

import numpy as np

sys.path.insert(0, "/opt/trn_rl_repo")

import concourse.bass as bass
import concourse.mybir as mybir
import concourse.tile as tile
from concourse import bacc
from concourse.bass_utils import run_bass_kernel_spmd

F32 = mybir.dt.float32
BF16 = mybir.dt.bfloat16
EXP = mybir.ActivationFunctionType.Exp

B, N_TOK, D, H_TOT, DH = 4, 2048, 1024, 16, 64
H = 8  # heads per core
HD = H * DH  # 512
SCALE = DH ** -0.5
N_CORES = 8


def emit_attention(ctx, tc, xt, wq, wk, wv, wo, out, rcp_dram, scale):
    """One core's shard. xt [D,N] f32; wq/wk/wv [D,HD]; wo [HD,D]; out [N,D];
    rcp_dram [H, N] f32 internal scratch for softmax-denominator broadcast."""
    nc = tc.nc
    D_, N = xt.shape
    HD_ = wq.shape[1]
    H_ = HD_ // 64
    KC = D_ // 128   # contraction chunks over model dim
    TC = N // 128    # token chunks (j-chunks)
    IB = N // 512    # 512-wide moving blocks
    IH = N // 1024   # 1024-wide i-halves
    MC = HD_ // 128  # chunks over per-core head dim
    NB = D_ // 512   # output column blocks

    p_tmp = ctx.enter_context(tc.tile_pool(name="tmp", bufs=3))
    p_xt = ctx.enter_context(tc.tile_pool(name="xt", bufs=KC))
    p_w = ctx.enter_context(tc.tile_pool(name="w", bufs=3 * KC))
    p_wo = ctx.enter_context(tc.tile_pool(name="wo", bufs=MC))
    p_qt = ctx.enter_context(tc.tile_pool(name="qt", bufs=MC))
    p_kt = ctx.enter_context(tc.tile_pool(name="kt", bufs=MC))
    p_v = ctx.enter_context(tc.tile_pool(name="v", bufs=TC))
    p_pt = ctx.enter_context(tc.tile_pool(name="pt", bufs=3))
    p_ot = ctx.enter_context(tc.tile_pool(name="ot", bufs=MC))
    p_nrm = ctx.enter_context(tc.tile_pool(name="nrm", bufs=1))
    p_stage = ctx.enter_context(tc.tile_pool(name="stage", bufs=2))

    # four 2-bank PSUM pools
    ps_sml = ctx.enter_context(tc.tile_pool(name="ps_sml", bufs=2, space="PSUM"))
    ps_o = ctx.enter_context(tc.tile_pool(name="ps_o", bufs=1, space="PSUM"))
    ps_qk = ctx.enter_context(tc.tile_pool(name="ps_qk", bufs=1, space="PSUM"))

    # ---- load + cast weights (q,k first: QK(0) prefix runs first) ----
    w_t = {}
    for nm, w_dram in (("q", wq), ("k", wk), ("v", wv)):
        for k in range(KC):
            t_in = p_tmp.tile([128, HD_], F32, name=f"wt_{nm}{k}", tag="tmp")
            nc.sync.dma_start(t_in[:], w_dram[k * 128 : (k + 1) * 128, :])
            t_bf = p_w.tile([128, HD_], BF16, name=f"w_{nm}{k}", tag="w")
            nc.vector.tensor_copy(t_bf[:], t_in[:])
            w_t[(nm, k)] = t_bf

    # ---- xT: load f32 halves, cast to resident bf16 [128, N] chunks ----
    xt_t = []
    for k in range(KC):
        t_bf = p_xt.tile([128, N], BF16, name=f"xt{k}", tag="xt")
        for hf in range(IH):
            t_in = p_tmp.tile([128, 1024], F32, name=f"xin{k}_{hf}", tag="xtmp")
            nc.sync.dma_start(
                t_in[:], xt[k * 128 : (k + 1) * 128, hf * 1024 : (hf + 1) * 1024]
            )
            nc.vector.tensor_copy(t_bf[:, hf * 1024 : (hf + 1) * 1024], t_in[:])
        xt_t.append(t_bf)

    wo_t = {}
    for kc in range(MC):
        t_in = p_tmp.tile([128, D_], F32, name=f"wot{kc}", tag="tmp")
        nc.sync.dma_start(t_in[:], wo[kc * 128 : (kc + 1) * 128, :])
        t_bf = p_wo.tile([128, D_], BF16, name=f"wo{kc}", tag="wo")
        nc.vector.tensor_copy(t_bf[:], t_in[:])
        wo_t[kc] = t_bf

    qt_tiles = [p_qt.tile([128, N], BF16, name=f"qt{m}", tag="qt") for m in range(MC)]
    kt_tiles = [p_kt.tile([128, N], BF16, name=f"kt{m}", tag="kt") for m in range(MC)]
    ot_tiles = [p_ot.tile([128, N], BF16, name=f"ot{m}", tag="ot") for m in range(MC)]
    v_tiles = []
    for tm in range(TC):
        vt = p_v.tile([128, H_ * 65], BF16, name=f"v{tm}", tag="v")
        nc.vector.memset(vt[:], 1.0)
        v_tiles.append(vt)

    def emit_qk(m, pool, tag):
        """Q^T/K^T chunk m: w stationary, xt moving; via `pool` [128,1024]."""
        for nm, out_tiles in (("q", qt_tiles), ("k", kt_tiles)):
            for half in range(IH):
                ps = pool.tile([128, 1024], F32, name=f"ps{nm}{m}_{half}", tag=tag)
                for k in range(KC):
                    for u in range(2):
                        ib = half * 2 + u
                        nc.tensor.matmul(
                            ps[:, u * 512 : (u + 1) * 512],
                            w_t[(nm, k)][:, m * 128 : (m + 1) * 128],
                            xt_t[k][:, ib * 512 : (ib + 1) * 512],
                            start=(k == 0),
                            stop=(k == KC - 1),
                        )
                nc.vector.tensor_copy(
                    out_tiles[m][:, half * 1024 : (half + 1) * 1024], ps[:]
                )

    def emit_v():
        """V natural [tokens, dh] -> v_aug tiles (ones col); filler pool."""
        for tp in range(TC // 2):
            ps = ps_qk.tile([128, 2 * HD_], F32, name=f"psv{tp}", tag="ps_qk")
            for k in range(KC):
                for u in range(2):
                    tm = tp * 2 + u
                    nc.tensor.matmul(
                        ps[:, u * HD_ : (u + 1) * HD_],
                        xt_t[k][:, tm * 128 : (tm + 1) * 128],
                        w_t[("v", k)][:],
                        start=(k == 0),
                        stop=(k == KC - 1),
                    )
            for u in range(2):
                tm = tp * 2 + u
                dst = v_tiles[tm][:].rearrange("p (h c) -> p h c", h=H_)[:, :, 0:64]
                src = ps[:, u * HD_ : (u + 1) * HD_].rearrange(
                    "p (h c) -> p h c", h=H_
                )
                nc.vector.tensor_copy(dst, src)

    def emit_head(h):
        mh, po = h // 2, (h % 2) * 64
        rsum = p_nrm.tile([1, N], F32, name=f"rsum{h}", tag="rsum")
        for ihalf in range(IH):
            i0 = ihalf * 1024
            pso = ps_o.tile([128, 1024], F32, name=f"pso{h}_{ihalf}", tag="ps_o")
            for jc in range(TC):
                kt_sl = kt_tiles[mh][po : po + 64, jc * 128 : (jc + 1) * 128]
                ptg = p_pt.tile([128, 1024], BF16, name=f"pt{h}_{ihalf}_{jc}", tag="pt")
                pss = ps_sml.tile(
                    [128, 1024], F32, name=f"pss{h}_{ihalf}_{jc}", tag="ps_sml"
                )
                for u in range(2):
                    nc.tensor.matmul(
                        pss[:, u * 512 : (u + 1) * 512],
                        kt_sl,
                        qt_tiles[mh][po : po + 64, i0 + u * 512 : i0 + (u + 1) * 512],
                        start=True,
                        stop=True,
                    )
                nc.scalar.activation(ptg[:], pss[:], EXP, scale=scale)
                for u in range(2):
                    nc.tensor.matmul(
                        pso[0:65, u * 512 : (u + 1) * 512],
                        v_tiles[jc][:, h * 65 : h * 65 + 65],
                        ptg[:, u * 512 : (u + 1) * 512],
                        start=(jc == 0),
                        stop=(jc == TC - 1),
                    )
            nc.vector.tensor_copy(rsum[:, i0 : i0 + 1024], pso[64:65, :])
            nc.vector.tensor_copy(
                ot_tiles[mh][po : po + 64, i0 : i0 + 1024], pso[0:64, :]
            )
        # recip -> DRAM bounce -> broadcast -> in-place normalize
        rcp = p_nrm.tile([1, N], F32, name=f"rcp{h}", tag="rcp")
        nc.vector.reciprocal_approx_fast(rcp[:], rsum[:])
        nc.sync.dma_start(rcp_dram[h : h + 1, :], rcp[:])
        bc = p_nrm.tile([128, N], F32, name=f"bc{h}", tag="bc")
        nc.sync.dma_start(
            bc[po : po + 64, :], rcp_dram[h : h + 1, :].to_broadcast((64, N))
        )
        nc.vector.tensor_tensor(
            ot_tiles[mh][po : po + 64, :],
            ot_tiles[mh][po : po + 64, :],
            bc[po : po + 64, :],
            op=mybir.AluOpType.mult,
        )

    # ---- schedule: QK0 dense prefix, then heads with filler work ----
    emit_qk(0, ps_sml, "ps_sml")
    emit_v()
    emit_head(0)
    emit_qk(1, ps_qk, "ps_qk")
    emit_head(1)
    emit_head(2)
    emit_qk(2, ps_qk, "ps_qk")
    emit_head(3)
    emit_head(4)
    emit_qk(3, ps_qk, "ps_qk")
    emit_head(5)
    emit_head(6)
    emit_head(7)

    # ---- output projection: out[t, dout] (ot stationary, wo moving) ----
    for tm in range(TC):
        stage = p_stage.tile([128, D_], F32, name=f"stg{tm}", tag="stage")
        for half in range(NB // 2):
            ps = ps_sml.tile([128, 1024], F32, name=f"psp{tm}_{half}", tag="ps_sml")
            for kc in range(MC):
                for u in range(2):
                    nb = half * 2 + u
                    nc.tensor.matmul(
                        ps[:, u * 512 : (u + 1) * 512],
                        ot_tiles[kc][:, tm * 128 : (tm + 1) * 128],
                        wo_t[kc][:, nb * 512 : (nb + 1) * 512],
                        start=(kc == 0),
                        stop=(kc == MC - 1),
                    )
            nc.vector.tensor_copy(
                stage[:, half * 1024 : (half + 1) * 1024], ps[:]
            )
        nc.sync.dma_start(out[tm * 128 : (tm + 1) * 128, :], stage[:])


_NC_CACHE = {}


def build_nc():
    if "nc" in _NC_CACHE:
        return _NC_CACHE["nc"]
    nc = bacc.Bacc("TRN2", target_bir_lowering=False, debug=False, num_devices=N_CORES)
    xt = nc.dram_tensor("xt", [D, N_TOK], F32, kind="ExternalInput")
    wq = nc.dram_tensor("wq", [D, HD], F32, kind="ExternalInput")
    wk = nc.dram_tensor("wk", [D, HD], F32, kind="ExternalInput")
    wv = nc.dram_tensor("wv", [D, HD], F32, kind="ExternalInput")
    wo = nc.dram_tensor("wo", [HD, D], F32, kind="ExternalInput")
    out = nc.dram_tensor("out", [N_TOK, D], F32, kind="ExternalOutput")
    rcp_d = nc.dram_tensor("rcp_d", [H, N_TOK], F32, kind="Internal")
    with tile.TileContext(nc) as tc:
        with ExitStack() as ctx:
            emit_attention(
                ctx, tc, xt.ap(), wq.ap(), wk.ap(), wv.ap(), wo.ap(), out.ap(),
                rcp_d.ap(), SCALE,
            )
    nc.compile()
    _NC_CACHE["nc"] = nc
    return nc


def kernel(x, Wq, Wk, Wv, Wo, bo, _trace=False, _trace_kwargs=None):
    assert x.shape == (B, N_TOK, D)
    nc = build_nc()
    in_maps = []
    for c in range(N_CORES):
        b, g = c // 2, c % 2
        in_maps.append(
            {
                "xt": np.ascontiguousarray(x[b].T).astype(np.float32),
                "wq": np.ascontiguousarray(Wq[:, g * HD : (g + 1) * HD]),
                "wk": np.ascontiguousarray(Wk[:, g * HD : (g + 1) * HD]),
                "wv": np.ascontiguousarray(Wv[:, g * HD : (g + 1) * HD]),
                "wo": np.ascontiguousarray(Wo[g * HD : (g + 1) * HD, :]),
            }
        )
    res = run_bass_kernel_spmd(
        nc,
        in_maps,
        core_ids=list(range(N_CORES)),
        trace=_trace,
        **(_trace_kwargs or {}),
    )
    out = np.empty((B, N_TOK, D), dtype=np.float32)
    for b in range(B):
        out[b] = res.results[2 * b]["out"] + res.results[2 * b + 1]["out"] + bo
    if _trace:
        kernel.last_results = res
    return out


# revision 15
# speedup vs baseline: 1.4661x; 1.0343x over previous
"""Multi-head attention (B=4, N=2048, D=1024, H=16, DH=64) on 8 TRN2 NeuronCores.

Sharding: core c <- (batch b = c//2, head-group g = c%2 of 8 heads).
  Each core computes its 8 heads' attention for its batch and the partial
  output projection (row-split Wo). Host sums the 2 partials per batch and
  adds the bias (the unshard step of tensor parallelism).

Device design (v4):
  - all matmul operands bf16, every matmul N=512 out (one PSUM bank/write);
    warm PE issues back-to-back at ~216ns with LDWEIGHTS pulled ahead.
  - heads processed in two 1024-wide i-half passes; PSUM split into four
    [128,1024] pools (2 banks each): scores ping/pong, PV accumulator, and
    a filler pool so V / later QK projections / output projection matmuls
    can run inside the ACT-bound heads phase's PE slack.
  - emission order = QK(m=0), head 0, V, QK1, H1, H2, QK2, H3, H4, QK3,
    H5, H6, H7, outproj; the Tile scheduler back-fills PE gaps with the
    lower-priority filler work while exp paces the pipeline.
  - softmax normalization off the critical path: ones-column rowsums ->
    fast reciprocal -> DRAM bounce -> 0-stride broadcast read -> in-place
    DVE multiply on the unnormalized O^T.
"""

import sys
from contextlib import ExitStack

import numpy as np

sys.path.insert(0, "/opt/trn_rl_repo")

import concourse.bass as bass
import concourse.mybir as mybir
import concourse.tile as tile
from concourse import bacc
from concourse.bass_utils import run_bass_kernel_spmd

F32 = mybir.dt.float32
BF16 = mybir.dt.bfloat16
EXP = mybir.ActivationFunctionType.Exp

B, N_TOK, D, H_TOT, DH = 4, 2048, 1024, 16, 64
H = 8  # heads per core
HD = H * DH  # 512
SCALE = DH ** -0.5
N_CORES = 8


def emit_attention(ctx, tc, xt, wq, wk, wv, wo, out, rcp_dram, scale):
    """One core's shard. xt [D,N] f32; wq/wk/wv [D,HD]; wo [HD,D]; out [N,D];
    rcp_dram [H, N] f32 internal scratch for softmax-denominator broadcast."""
    nc = tc.nc
    D_, N = xt.shape
    HD_ = wq.shape[1]
    H_ = HD_ // 64
    KC = D_ // 128   # contraction chunks over model dim
    TC = N // 128    # token chunks (j-chunks)
    IB = N // 512    # 512-wide moving blocks
    IH = N // 1024   # 1024-wide i-halves
    MC = HD_ // 128  # chunks over per-core head dim
    NB = D_ // 512   # output column blocks

    p_tmp = ctx.enter_context(tc.tile_pool(name="tmp", bufs=3))
    p_xt = ctx.enter_context(tc.tile_pool(name="xt", bufs=KC))
    p_w = ctx.enter_context(tc.tile_pool(name="w", bufs=3 * KC))
    p_wo = ctx.enter_context(tc.tile_pool(name="wo", bufs=MC))
    p_qt = ctx.enter_context(tc.tile_pool(name="qt", bufs=MC))
    p_kt = ctx.enter_context(tc.tile_pool(name="kt", bufs=MC))
    p_v = ctx.enter_context(tc.tile_pool(name="v", bufs=TC))
    p_pt = ctx.enter_context(tc.tile_pool(name="pt", bufs=3))
    p_ot = ctx.enter_context(tc.tile_pool(name="ot", bufs=MC))
    p_nrm = ctx.enter_context(tc.tile_pool(name="nrm", bufs=1))
    p_stage = ctx.enter_context(tc.tile_pool(name="stage", bufs=2))

    # four 2-bank PSUM pools
    ps_sml = ctx.enter_context(tc.tile_pool(name="ps_sml", bufs=2, space="PSUM"))
    ps_o = ctx.enter_context(tc.tile_pool(name="ps_o", bufs=1, space="PSUM"))
    ps_qk = ctx.enter_context(tc.tile_pool(name="ps_qk", bufs=1, space="PSUM"))

    # ---- load + cast weights (q,k first: QK(0) prefix runs first) ----
    dma_engs = [nc.sync, nc.gpsimd]
    w_t = {}
    for wi, (nm, w_dram) in enumerate((("q", wq), ("k", wk), ("v", wv))):
        for k in range(KC):
            t_in = p_tmp.tile([128, HD_], F32, name=f"wt_{nm}{k}", tag="tmp")
            dma_engs[k % 2].dma_start(t_in[:], w_dram[k * 128 : (k + 1) * 128, :])
            t_bf = p_w.tile([128, HD_], BF16, name=f"w_{nm}{k}", tag="w")
            nc.vector.tensor_copy(t_bf[:], t_in[:])
            w_t[(nm, k)] = t_bf

    # ---- xT: load f32 halves, cast to resident bf16 [128, N] chunks ----
    xt_t = []
    for k in range(KC):
        t_bf = p_xt.tile([128, N], BF16, name=f"xt{k}", tag="xt")
        for hf in range(IH):
            t_in = p_tmp.tile([128, 1024], F32, name=f"xin{k}_{hf}", tag="xtmp")
            dma_engs[(k + hf) % 2].dma_start(
                t_in[:], xt[k * 128 : (k + 1) * 128, hf * 1024 : (hf + 1) * 1024]
            )
            nc.vector.tensor_copy(t_bf[:, hf * 1024 : (hf + 1) * 1024], t_in[:])
        xt_t.append(t_bf)

    wo_t = {}
    for kc in range(MC):
        t_in = p_tmp.tile([128, D_], F32, name=f"wot{kc}", tag="tmp")
        nc.sync.dma_start(t_in[:], wo[kc * 128 : (kc + 1) * 128, :])
        t_bf = p_wo.tile([128, D_], BF16, name=f"wo{kc}", tag="wo")
        nc.vector.tensor_copy(t_bf[:], t_in[:])
        wo_t[kc] = t_bf

    qt_tiles = [p_qt.tile([128, N], BF16, name=f"qt{m}", tag="qt") for m in range(MC)]
    kt_tiles = [p_kt.tile([128, N], BF16, name=f"kt{m}", tag="kt") for m in range(MC)]
    ot_tiles = [p_ot.tile([128, N], BF16, name=f"ot{m}", tag="ot") for m in range(MC)]
    v_tiles = []
    for tm in range(TC):
        vt = p_v.tile([128, H_ * 65], BF16, name=f"v{tm}", tag="v")
        nc.vector.memset(vt[:], 1.0)
        v_tiles.append(vt)

    def emit_qk(m, pool, tag):
        """Q^T/K^T chunk m: w stationary, xt moving; via `pool` [128,1024]."""
        for step in qk_steps(m, pool, tag):
            step()

    def qk_steps(m, pool, tag):
        """Yield zero-arg closures emitting QK chunk m piecewise: each step is
        one (k, u) matmul or the trailing evacuation copy of a half."""
        for nm, out_tiles in (("q", qt_tiles), ("k", kt_tiles)):
            for half in range(IH):
                ps = [None]

                def mk_mm(nm, half, k, u, ps):
                    def go():
                        if ps[0] is None:
                            ps[0] = pool.tile(
                                [128, 1024], F32, name=f"ps{nm}{m}_{half}", tag=tag
                            )
                        ib = half * 2 + u
                        nc.tensor.matmul(
                            ps[0][:, u * 512 : (u + 1) * 512],
                            w_t[(nm, k)][:, m * 128 : (m + 1) * 128],
                            xt_t[k][:, ib * 512 : (ib + 1) * 512],
                            start=(k == 0),
                            stop=(k == KC - 1),
                        )
                    return go

                def mk_evac(nm, half, out_tiles, ps):
                    def go():
                        nc.vector.tensor_copy(
                            out_tiles[m][:, half * 1024 : (half + 1) * 1024],
                            ps[0][:],
                        )
                    return go

                for k in range(KC):
                    for u in range(2):
                        yield mk_mm(nm, half, k, u, ps)
                yield mk_evac(nm, half, out_tiles, ps)

    def emit_v():
        """V natural [tokens, dh] -> v_aug tiles (ones col); filler pool."""
        for tp in range(TC // 2):
            ps = ps_qk.tile([128, 2 * HD_], F32, name=f"psv{tp}", tag="ps_qk")
            for k in range(KC):
                for u in range(2):
                    tm = tp * 2 + u
                    nc.tensor.matmul(
                        ps[:, u * HD_ : (u + 1) * HD_],
                        xt_t[k][:, tm * 128 : (tm + 1) * 128],
                        w_t[("v", k)][:],
                        start=(k == 0),
                        stop=(k == KC - 1),
                    )
            for u in range(2):
                tm = tp * 2 + u
                dst = v_tiles[tm][:].rearrange("p (h c) -> p h c", h=H_)[:, :, 0:64]
                src = ps[:, u * HD_ : (u + 1) * HD_].rearrange(
                    "p (h c) -> p h c", h=H_
                )
                nc.vector.tensor_copy(dst, src)

    def emit_head(h, filler=None):
        mh, po = h // 2, (h % 2) * 64
        rsum = p_nrm.tile([1, N], F32, name=f"rsum{h}", tag="rsum")
        for ihalf in range(IH):
            i0 = ihalf * 1024
            pso = ps_o.tile([128, 1024], F32, name=f"pso{h}_{ihalf}", tag="ps_o")
            for jc in range(TC):
                if filler is not None:
                    step = next(filler, None)
                    if step is not None:
                        step()
                kt_sl = kt_tiles[mh][po : po + 64, jc * 128 : (jc + 1) * 128]
                ptg = p_pt.tile([128, 1024], BF16, name=f"pt{h}_{ihalf}_{jc}", tag="pt")
                pss = ps_sml.tile(
                    [128, 1024], F32, name=f"pss{h}_{ihalf}_{jc}", tag="ps_sml"
                )
                for u in range(2):
                    nc.tensor.matmul(
                        pss[:, u * 512 : (u + 1) * 512],
                        kt_sl,
                        qt_tiles[mh][po : po + 64, i0 + u * 512 : i0 + (u + 1) * 512],
                        start=True,
                        stop=True,
                    )
                nc.scalar.activation(ptg[:], pss[:], EXP, scale=scale)
                for u in range(2):
                    nc.tensor.matmul(
                        pso[0:65, u * 512 : (u + 1) * 512],
                        v_tiles[jc][:, h * 65 : h * 65 + 65],
                        ptg[:, u * 512 : (u + 1) * 512],
                        start=(jc == 0),
                        stop=(jc == TC - 1),
                    )
            nc.vector.tensor_copy(rsum[:, i0 : i0 + 1024], pso[64:65, :])
            nc.vector.tensor_copy(
                ot_tiles[mh][po : po + 64, i0 : i0 + 1024], pso[0:64, :]
            )
        # recip -> DRAM bounce -> broadcast -> in-place normalize
        rcp = p_nrm.tile([1, N], F32, name=f"rcp{h}", tag="rcp")
        nc.vector.reciprocal_approx_fast(rcp[:], rsum[:])
        nc.sync.dma_start(rcp_dram[h : h + 1, :], rcp[:])
        bc = p_nrm.tile([128, N], F32, name=f"bc{h}", tag="bc")
        nc.sync.dma_start(
            bc[po : po + 64, :], rcp_dram[h : h + 1, :].to_broadcast((64, N))
        )
        nc.vector.tensor_tensor(
            ot_tiles[mh][po : po + 64, :],
            ot_tiles[mh][po : po + 64, :],
            bc[po : po + 64, :],
            op=mybir.AluOpType.mult,
        )

    # ---- schedule: QK0 dense prefix, then heads with filler work ----
    emit_qk(0, ps_sml, "ps_sml")
    emit_v()
    qk1 = qk_steps(1, ps_qk, "ps_qk")
    emit_head(0, qk1)
    emit_head(1, qk1)
    for step in qk1:
        step()
    qk2 = qk_steps(2, ps_qk, "ps_qk")
    emit_head(2, qk2)
    emit_head(3, qk2)
    for step in qk2:
        step()
    qk3 = qk_steps(3, ps_qk, "ps_qk")
    emit_head(4, qk3)
    emit_head(5, qk3)
    for step in qk3:
        step()
    emit_head(6)
    emit_head(7)

    # ---- output projection: out[t, dout] (ot stationary, wo moving) ----
    for tm in range(TC):
        stage = p_stage.tile([128, D_], F32, name=f"stg{tm}", tag="stage")
        for half in range(NB // 2):
            ps = ps_sml.tile([128, 1024], F32, name=f"psp{tm}_{half}", tag="ps_sml")
            for kc in range(MC):
                for u in range(2):
                    nb = half * 2 + u
                    nc.tensor.matmul(
                        ps[:, u * 512 : (u + 1) * 512],
                        ot_tiles[kc][:, tm * 128 : (tm + 1) * 128],
                        wo_t[kc][:, nb * 512 : (nb + 1) * 512],
                        start=(kc == 0),
                        stop=(kc == MC - 1),
                    )
            nc.vector.tensor_copy(
                stage[:, half * 1024 : (half + 1) * 1024], ps[:]
            )
        nc.sync.dma_start(out[tm * 128 : (tm + 1) * 128, :], stage[:])


_NC_CACHE = {}


def build_nc():
    if "nc" in _NC_CACHE:
        return _NC_CACHE["nc"]
    nc = bacc.Bacc("TRN2", target_bir_lowering=False, debug=False, num_devices=N_CORES)
    xt = nc.dram_tensor("xt", [D, N_TOK], F32, kind="ExternalInput")
    wq = nc.dram_tensor("wq", [D, HD], F32, kind="ExternalInput")
    wk = nc.dram_tensor("wk", [D, HD], F32, kind="ExternalInput")
    wv = nc.dram_tensor("wv", [D, HD], F32, kind="ExternalInput")
    wo = nc.dram_tensor("wo", [HD, D], F32, kind="ExternalInput")
    out = nc.dram_tensor("out", [N_TOK, D], F32, kind="ExternalOutput")
    rcp_d = nc.dram_tensor("rcp_d", [H, N_TOK], F32, kind="Internal")
    with tile.TileContext(nc) as tc:
        with ExitStack() as ctx:
            emit_attention(
                ctx, tc, xt.ap(), wq.ap(), wk.ap(), wv.ap(), wo.ap(), out.ap(),
                rcp_d.ap(), SCALE,
            )
    nc.compile()
    _NC_CACHE["nc"] = nc
    return nc


def kernel(x, Wq, Wk, Wv, Wo, bo, _trace=False, _trace_kwargs=None):
    assert x.shape == (B, N_TOK, D)
    nc = build_nc()
    in_maps = []
    for c in range(N_CORES):
        b, g = c // 2, c % 2
        in_maps.append(
            {
                "xt": np.ascontiguousarray(x[b].T).astype(np.float32),
                "wq": np.ascontiguousarray(Wq[:, g * HD : (g + 1) * HD]),
                "wk": np.ascontiguousarray(Wk[:, g * HD : (g + 1) * HD]),
                "wv": np.ascontiguousarray(Wv[:, g * HD : (g + 1) * HD]),
                "wo": np.ascontiguousarray(Wo[g * HD : (g + 1) * HD, :]),
            }
        )
    res = run_bass_kernel_spmd(
        nc,
        in_maps,
        core_ids=list(range(N_CORES)),
        trace=_trace,
        **(_trace_kwargs or {}),
    )
    out = np.empty((B, N_TOK, D), dtype=np.float32)
    for b in range(B):
        out[b] = res.results[2 * b]["out"] + res.results[2 * b + 1]["out"] + bo
    if _trace:
        kernel.last_results = res
    return out
